# revision 13
# baseline (speedup 1.0000x reference)
"""Sparse-attention Trainium2 kernel (nn_Attention_81398220193933).

Strategy (tensor-parallel over heads, 2 heads per NeuronCore):
  - Host pre-lays-out per-core tensors:
      qT  [B, 128, S]  bf16 : rows 0:64 = headA Q^T / sqrt(dh), rows 64:128 = headB
      kT  [B, 128, S]  bf16 : same for K^T
      vE  [B, 128, 8, 130] bf16 : per k-tile t, partition p = key position t*128+p,
           cols [0:64]=V_A*emb, [64]=emb, [65:129]=V_B*emb, [129]=emb
           where emb[b,k] = exp(bias[k]) * (k < seq_len[b]) (all-valid if seq_len==0).
    Folding the additive key bias + mask multiplicatively into V makes the
    softmax mask/bias free on-device and lets fully-masked k-tiles be skipped.
  - Device, per batch b and key-tile t (Kb = ceil(seq_len/128) tiles):
      scores^T [k=128, q=1024] = K_tile^T.T @ Q^T  per head (2 PSUM-bank MMs)
      W^T = exp(scores^T) on ScalarE (PSUM -> SBUF, bf16)
      outT [65, q] += V_tile(+emb col).T @ W^T   accumulated over t in PSUM.
        The transposed A-V orientation makes V the stationary operand: one
        LDWEIGHTS + two 512-row matmuls per (tile, head) instead of eight
        128-query chunks, and row 64 accumulates the softmax denominator.
      Epilogue per (batch, head): DVE copies outT [65, 1024] PSUM -> SBUF,
      DMA to HBM. The numerator/denominator DIVISION happens on the host
      (out = o[0:64]/o[64]) -- the device ships unnormalized sums.
  - Softmax max-subtraction is unnecessary: logits are O(+-6) and masked keys
    contribute exactly zero through emb; a fully-masked row degenerates to
    softmax over all keys exactly like the jax reference (the -1e12 shift
    cancels there).
  - ScalarE is the bound engine (~80us of exp); the schedule keeps it fed:
      * PSUM: 2 score slots (4 banks) + 2 A-V accumulator slots (4 banks).
        QK(s+1) is emitted with exp(s) and reuses the slot exp(s-1) freed;
        double-buffered accumulators let the next head's A-V chain start
        while DVE drains the previous head's.
      * the PE HAM clock-gate (binary 1.2/2.4 GHz, ~3.4us activity window)
        is warmed by a junk-matmul burst sourced from the framework const
        region (no DMA/memset dependency) writing into the first score
        tile's partition 0 (overwritten by the real QK), and kept warm by
        one such junk matmul per key tile.
"""

import numpy as np
import ml_dtypes

import concourse.bass as bass
import concourse.mybir as mybir
import concourse.tile as tile
from concourse import bacc
from concourse.bass_utils import run_bass_kernel_spmd


def _dedup_ldweights(nc):
    """Remove redundant InstLdweights from the PE stream.

    The tile legalizer pairs one LDWEIGHTS with EVERY matmul, so the two
    512-col matmuls that share a stationary tile reload it back-to-back
    (~113ns each, fully serialized on the PE queue -- ~16us over the
    kernel).  An LDW is redundant when the weights signature (access
    pattern + offset + tile_position) matches the live weights in every
    array row-strip it covers and it carries no materialized semaphore
    wait.  Nothing references LDWs as a dependency (verified), so removal
    is a pure list edit."""
    from concourse.mybir import InstLdweights, InstMatmult

    for f in nc.m.functions:
        for bb in f.blocks:
            insts = list(bb.instructions)
            live = []  # list of (row_lo, row_hi, sig)
            keep = []
            removed = 0
            for i in insts:
                if isinstance(i, InstLdweights):
                    ap = i.ins[0]
                    tp = i.tile_position or (0, 0)
                    nrows = ap.ap[0][1]
                    lo, hi = tp[0], tp[0] + nrows
                    sig = (str(ap.ap), ap.offset, str(ap.dtype), tp,
                           str(i.perf_mode), str(i.is_transpose))
                    cover = [e for e in live if not (e[1] <= lo or e[0] >= hi)]
                    same = (len(cover) == 1 and cover[0][0] == lo
                            and cover[0][1] == hi and cover[0][2] == sig)
                    has_wait = (i.sync_info is not None
                                and len(i.sync_info.on_wait) > 0)
                    if same and not has_wait:
                        removed += 1
                        continue
                    live = [e for e in live if (e[1] <= lo or e[0] >= hi)]
                    live.append((lo, hi, sig))
                keep.append(i)
            if removed:
                del bb.instructions[:]
                for i in keep:
                    bb.instructions.append(i)

B = 8
S = 1024
UNITS = 1024
H = 16
DH = 64
N_CORES = 8
KT = S // 128  # max key tiles per batch

BF16 = mybir.dt.bfloat16
F32 = mybir.dt.float32


def _build_nc(kbs):
    """Build the SPMD Bass program. kbs: per-batch number of 128-key tiles."""
    nc = bacc.Bacc("TRN2", target_bir_lowering=False, debug=False,
                   num_devices=N_CORES)
    qT = nc.dram_tensor("qt", [B, 128, S], BF16, kind="ExternalInput").ap()
    kT = nc.dram_tensor("kt", [B, 128, S], BF16, kind="ExternalInput").ap()
    vE = nc.dram_tensor("vt", [B, 128, KT, 256], BF16, kind="ExternalInput").ap()
    # unnormalized outputs: per (b, head) [65, S]; row 64 = denominator.
    # Shipped bf16 (halves the output DMA); the host divides in f32.
    o = nc.dram_tensor("o", [B, 2, 65, S], BF16, kind="ExternalOutput").ap()

    cb = nc.const_aps.aps[(BF16, 1.0)]  # [128, 1] framework const

    with tile.TileContext(nc) as tc:
        with (
            tc.tile_pool(name="qk", bufs=2) as qk_pool,
            tc.tile_pool(name="v", bufs=2) as v_pool,
            tc.tile_pool(name="w", bufs=32) as w_pool,
            tc.tile_pool(name="ot", bufs=4) as o_pool,
            tc.tile_pool(name="sc", bufs=2, space="PSUM") as sc_pool,
            tc.tile_pool(name="acc", bufs=2, space="PSUM") as acc_pool,
        ):
            def junk(sc_t, n, width=256):
                """n PE warm-keeper matmuls with zero PSUM footprint: const
                [128,1] bf16 broadcast to a full 128x128 stationary tile and
                a [128,width] moving tile (the HAM activity monitor only
                counts real array occupancy), written into a score tile that
                the next QK overwrites."""
                lhsT = bass.AP(tensor=cb.tensor, offset=cb.offset,
                               ap=[cb.ap[0], [0, 128]])
                rhs = bass.AP(tensor=cb.tensor, offset=cb.offset,
                              ap=[cb.ap[0], [0, width]])
                for _ in range(n):
                    nc.tensor.matmul(sc_t[:, 0:width], lhsT=lhsT,
                                     rhs=rhs, start=True, stop=True,
                                     skip_group_check=True)

            # ACT exp table preload (~2.7us) while the preamble/DMAs fly.
            wexp = qk_pool.tile([1, 8], F32, tag="wexp", name="wexp", bufs=1)
            nc.gpsimd.memset(wexp[:], 0.0)
            nc.scalar.activation(wexp[:], wexp[:],
                                 mybir.ActivationFunctionType.Exp)

            # Load every batch's inputs up front (fits easily in SBUF) so no
            # QK phase ever waits on DMA. First batch's K/Q head-A halves go
            # first so the first QK can start ~1us earlier; V tiles are only
            # needed one batch later.
            # First batch small, then largest-first, smallest last (short
            # tail after the final exp).
            srt = sorted(range(B), key=lambda i: -kbs[i])
            order = [srt[-2]] + srt[:-2] + [srt[-1]]
            b0 = order[0]
            qts, kts, vts = {}, {}, {}
            for b in order:
                qts[b] = qk_pool.tile([128, S], BF16, tag=f"qt{b}",
                                      name=f"qt{b}", bufs=1)
                # K only needs the valid key columns
                kts[b] = qk_pool.tile([128, 128 * kbs[b]], BF16, tag=f"kt{b}",
                                      name=f"kt{b}", bufs=1)
            for b in order:
                vts[b] = v_pool.tile([128, kbs[b], 256], BF16, tag=f"vt{b}",
                                     name=f"vt{b}", bufs=1)
            kb0 = kbs[b0]
            nc.sync.dma_start(out=kts[b0][0:64, :], in_=kT[b0, 0:64, :128 * kb0])
            nc.sync.dma_start(out=qts[b0][0:64, :], in_=qT[b0, 0:64, :])
            nc.sync.dma_start(out=kts[b0][64:128, :],
                              in_=kT[b0, 64:128, :128 * kb0])
            nc.sync.dma_start(out=qts[b0][64:128, :], in_=qT[b0, 64:128, :])
            # Interleave: batch b+1's K/Q, then batch b's V — each batch's V
            # arrives before its A-V drip starts, K/Q before its QK phase.
            for i, b in enumerate(order[1:]):
                nc.sync.dma_start(out=kts[b][:], in_=kT[b, :, :128 * kbs[b]])
                nc.sync.dma_start(out=qts[b][:], in_=qT[b])
                prev = order[i]
                nc.sync.dma_start(out=vts[prev][:],
                                  in_=vE[prev, :, :kbs[prev], :])
            bl = order[-1]
            nc.sync.dma_start(out=vts[bl][:], in_=vE[bl, :, :kbs[bl], :])

            # Flat substep stream: one substep per (batch, key-tile, head).
            # Normal batches tile-major (t,h); the last batch head-major so
            # head A's A-V (which needs every A exp) overlaps head B's exp
            # phase instead of extending the kernel tail.
            recs = {}
            subs = []
            for bi, b in enumerate(order):
                kb = kbs[b]
                last = bi == len(order) - 1
                recs[b] = {"b": b, "kb": kb, "wts": [[None] * kb, [None] * kb],
                           "vt": vts[b], "last": last}
                if last:
                    sl = [(t, h) for h in range(2) for t in range(kb)]
                else:
                    sl = [(t, h) for t in range(kb) for h in range(2)]
                subs.extend((b, t, h) for (t, h) in sl)

            def emit_qk(s, warm=2, warm_width=256):
                b, t, h = s
                base = 64 * h
                sc = sc_pool.tile([128, S], F32, tag="sc", name="sc")
                junk(sc, warm, warm_width)
                for qc in range(2):
                    nc.tensor.matmul(
                        sc[:, qc * 512:(qc + 1) * 512],
                        lhsT=kts[b][base:base + 64, t * 128:(t + 1) * 128],
                        rhs=qts[b][base:base + 64, qc * 512:(qc + 1) * 512],
                        start=True, stop=True,
                    )
                return sc

            # Schraudolph constants for the DVE exp approximation:
            # bf16 bits of exp(s) ~= int16(A*s + B); A = 128/ln(2), B centers
            # the (1+f)/2^f mantissa error (max ~4% on weights, ~1% on the
            # softmax-averaged output). ScalarE alone is the exp bottleneck
            # (~80us); shifting every 3rd tile to the otherwise-idle DVE
            # takes ~20us off the critical path.
            EXP_A = 184.6649652337873
            EXP_B = 16250.5

            def emit_exp(s, sc, on_dve):
                b, t, h = s
                wt = w_pool.tile([128, S], BF16, tag="w", name=f"w{b}_{t}_{h}")
                if on_dve:
                    nc.vector.tensor_scalar(
                        wt[:].bitcast(mybir.dt.int16), sc[:],
                        EXP_A, EXP_B,
                        mybir.AluOpType.mult, mybir.AluOpType.add)
                else:
                    nc.scalar.activation(wt[:], sc[:],
                                         mybir.ActivationFunctionType.Exp)
                recs[b]["wts"][h][t] = wt

            def emit_av(p, h, t):
                """One A-V accumulation step: outT[65, S] += V_t.T @ W_t^T
                (two 512-col matmuls, one per PSUM bank half). Stationary is
                the 65 used V columns only (LDWEIGHTS cost scales with
                stationary columns: 65 -> ~54ns vs 128 -> ~107ns)."""
                if t == 0:
                    p["acc"] = acc_pool.tile([128, 2, 512], F32, tag="acc",
                                             name=f"acc{p['b']}_{h}")
                kb = p["kb"]
                for half in range(2):
                    nc.tensor.matmul(
                        p["acc"][0:65, half, :],
                        lhsT=p["vt"][:, t, h * 128:h * 128 + 65],
                        rhs=p["wts"][h][t][:, half * 512:(half + 1) * 512],
                        start=(t == 0), stop=(t == kb - 1),
                    )
                if t == kb - 1:
                    epilogue(p, h)

            epi_no = [0]

            def epilogue(p, h):
                """Ship head h's unnormalized sums: PSUM -> SBUF(bf16) -> HBM.
                Copied and DMA'd in halves so the first DMA overlaps the
                second copy. Heads alternate between ScalarE and VectorE so
                neither exp engine eats the whole ~19us of copy work."""
                ot = o_pool.tile([65, 2, 512], BF16, tag="ot", name="ot")
                ov = o[p["b"], h]
                on_act = epi_no[0] % 2 == 0
                epi_no[0] += 1
                for c in range(2):
                    if on_act:
                        nc.scalar.copy(ot[:, c, :], p["acc"][0:65, c, :])
                    else:
                        nc.vector.tensor_scalar_mul(ot[:, c, :],
                                                    p["acc"][0:65, c, :], 1.0)
                    nc.sync.dma_start(out=ov[:, c * 512:(c + 1) * 512],
                                      in_=ot[:, c, :])

            # Global stream. exp(s) is emitted together with QK(s+1): with
            # the 3-deep score pool, QK(s+1) reuses a slot freed 1.5 tiles
            # ago, so the PE runs ahead and ScalarE never waits. A-V steps
            # of finished batches drip out at a bounded per-window rate so
            # they never pile up in front of the next QK pair.
            avq = []  # (batch record, head, t) FIFO
            total_steps = sum(kbs)
            kb0 = kbs[order[0]]
            # Exps on different engines run in parallel (independent score/
            # weight tiles), so strict ACT/DVE alternation doubles the exp
            # throughput. Use it where the stream is exp-paced (the first
            # two batches, before any A-V work exists, and the last batch's
            # tail); elsewhere the PE paces and a 2:1 split keeps DVE free
            # for the epilogue copies.
            nfirst2 = 2 * (kbs[order[0]] + kbs[order[1]])
            nlast = 2 * kbs[order[-1]]
            def dve_pick(i):
                if i < nfirst2 or i >= len(subs) - nlast:
                    return i % 2 == 1
                return i % 7 in (1, 3, 5)
            step_no = 0
            first_sc = emit_qk(subs[0], warm=10, warm_width=384)
            pending_sc = first_sc
            for i, s in enumerate(subs):
                b, t, h = s
                rec = recs[b]
                kb, last = rec["kb"], rec["last"]
                emit_exp(s, pending_sc, on_dve=dve_pick(i))
                if i + 1 < len(subs):
                    # first batch has no A-V drip yet: extra junk keeps the
                    # PE duty high enough that HAM doesn't re-throttle
                    # (removing it measured a better mean but a worse
                    # max-core: the ramp re-throttle returns on some cores)
                    pending_sc = emit_qk(subs[i + 1],
                                         warm=3 if i < 2 * kb0 else 0,
                                         warm_width=256)
                if last and h == 0 and t == kb - 1:
                    # head A complete: its A-V can interleave from here
                    avq.extend((rec, 0, tt) for tt in range(kb))
                step_no += 0 if h else 1
                rem = max(1, total_steps - step_no)
                rate = -(-len(avq) // min(rem, 8))
                cap = 6 if rem <= 4 else 3
                if h == 1 or last:
                    for _ in range(min(rate, cap)):
                        if avq:
                            emit_av(*avq.pop(0))
                if h == 1 and t == kb - 1:
                    if last:
                        avq.extend((rec, 1, tt) for tt in range(kb))
                    else:
                        avq.extend((rec, hh, tt)
                                   for hh in range(2) for tt in range(kb))

            while avq:
                emit_av(*avq.pop(0))
    _dedup_ldweights(nc)
    nc.compile()
    return nc


_NC_CACHE = {}


def _get_nc(kbs):
    key = tuple(kbs)
    if key not in _NC_CACHE:
        _NC_CACHE[key] = _build_nc(key)
    return _NC_CACHE[key]


def kernel(memory, query, b, seq_len):
    memory = np.asarray(memory)
    query = np.asarray(query)
    bias = np.asarray(b, dtype=np.float32)
    seq_len = np.asarray(seq_len).reshape(-1).astype(np.int64)

    sl = seq_len.copy()
    kbs = [int(min(KT, max(1, -(-int(s) // 128)))) if s > 0 else KT for s in sl]

    # emb[b, k] = exp(bias[k]) * valid; fully-masked batch -> plain softmax
    pos = np.arange(S)[None, :]
    valid = (pos < sl[:, None]) | (sl[:, None] == 0)
    emb = np.exp(bias)[None, :] * valid.astype(np.float32)  # [B, S]

    qh = (query.astype(np.float32) * (DH ** -0.5)).reshape(B, S, H, DH)
    kh = memory[:, :, :UNITS].astype(np.float32).reshape(B, S, H, DH)
    vh = memory[:, :, UNITS:].astype(np.float32).reshape(B, S, H, DH)
    vh = vh * emb[:, :, None, None]  # [B, S, H, DH] value rows pre-masked

    bf = ml_dtypes.bfloat16
    # [B, S, H, DH] -> [B, H, DH, S] transposed layouts
    qTfull = np.ascontiguousarray(qh.transpose(0, 2, 3, 1)).astype(bf)
    kTfull = np.ascontiguousarray(kh.transpose(0, 2, 3, 1)).astype(bf)
    # [B, S, H, DH] -> [B, (t p), H, DH] -> [B, 128, KT, H, DH]
    vtiles = np.ascontiguousarray(
        vh.reshape(B, KT, 128, H, DH).transpose(0, 2, 1, 3, 4)).astype(bf)
    embt = np.ascontiguousarray(
        emb.reshape(B, KT, 128).transpose(0, 2, 1)).astype(bf)  # [B, 128, KT]

    in_maps = []
    for c in range(N_CORES):
        hA, hB = 2 * c, 2 * c + 1
        qTc = np.concatenate([qTfull[:, hA], qTfull[:, hB]], axis=1)
        kTc = np.concatenate([kTfull[:, hA], kTfull[:, hB]], axis=1)
        vEc = np.zeros((B, 128, KT, 256), dtype=bf)
        vEc[..., 0:64] = vtiles[:, :, :, hA, :]
        vEc[..., 64] = embt
        vEc[..., 128:192] = vtiles[:, :, :, hB, :]
        vEc[..., 192] = embt
        in_maps.append({
            "qt": np.ascontiguousarray(qTc),
            "kt": np.ascontiguousarray(kTc),
            "vt": np.ascontiguousarray(vEc),
        })

    nc = _get_nc(kbs)
    res = run_bass_kernel_spmd(nc, in_maps, core_ids=list(range(N_CORES)))

    out = np.empty((B, S, UNITS), dtype=np.float32)
    for c in range(N_CORES):
        oc = np.asarray(res.results[c]["o"], dtype=np.float32)  # [B,2,65,S]
        num = oc[:, :, 0:64, :]                      # [B, 2, 64, S]
        den = oc[:, :, 64:65, :]                     # [B, 2, 1, S]
        core = (num / den).transpose(0, 3, 1, 2)     # [B, S, 2, 64]
        out[:, :, 128 * c:128 * (c + 1)] = core.reshape(B, S, 128)
    return out



# revision 17
# speedup vs baseline: 1.0170x; 1.0170x over previous
"""Sparse-attention Trainium2 kernel (nn_Attention_81398220193933).

Strategy (tensor-parallel over heads, 2 heads per NeuronCore):
  - Host pre-lays-out per-core tensors:
      qT  [B, 128, S]  bf16 : rows 0:64 = headA Q^T / sqrt(dh), rows 64:128 = headB
      kT  [B, 128, S]  bf16 : same for K^T
      vE  [B, 128, 8, 130] bf16 : per k-tile t, partition p = key position t*128+p,
           cols [0:64]=V_A*emb, [64]=emb, [65:129]=V_B*emb, [129]=emb
           where emb[b,k] = exp(bias[k]) * (k < seq_len[b]) (all-valid if seq_len==0).
    Folding the additive key bias + mask multiplicatively into V makes the
    softmax mask/bias free on-device and lets fully-masked k-tiles be skipped.
  - Device, per batch b and key-tile t (Kb = ceil(seq_len/128) tiles):
      scores^T [k=128, q=1024] = K_tile^T.T @ Q^T  per head (2 PSUM-bank MMs)
      W^T = exp(scores^T) on ScalarE (PSUM -> SBUF, bf16)
      outT [65, q] += V_tile(+emb col).T @ W^T   accumulated over t in PSUM.
        The transposed A-V orientation makes V the stationary operand: one
        LDWEIGHTS + two 512-row matmuls per (tile, head) instead of eight
        128-query chunks, and row 64 accumulates the softmax denominator.
      Epilogue per (batch, head): DVE copies outT [65, 1024] PSUM -> SBUF,
      DMA to HBM. The numerator/denominator DIVISION happens on the host
      (out = o[0:64]/o[64]) -- the device ships unnormalized sums.
  - Softmax max-subtraction is unnecessary: logits are O(+-6) and masked keys
    contribute exactly zero through emb; a fully-masked row degenerates to
    softmax over all keys exactly like the jax reference (the -1e12 shift
    cancels there).
  - ScalarE is the bound engine (~80us of exp); the schedule keeps it fed:
      * PSUM: 2 score slots (4 banks) + 2 A-V accumulator slots (4 banks).
        QK(s+1) is emitted with exp(s) and reuses the slot exp(s-1) freed;
        double-buffered accumulators let the next head's A-V chain start
        while DVE drains the previous head's.
      * the PE HAM clock-gate (binary 1.2/2.4 GHz, ~3.4us activity window)
        is warmed by a junk-matmul burst sourced from the framework const
        region (no DMA/memset dependency) writing into the first score
        tile's partition 0 (overwritten by the real QK), and kept warm by
        one such junk matmul per key tile.
"""

import numpy as np
import ml_dtypes

import concourse.bass as bass
import concourse.mybir as mybir
import concourse.tile as tile
from concourse import bacc
from concourse.bass_utils import run_bass_kernel_spmd


def _dedup_ldweights(nc):
    """Remove redundant InstLdweights from the PE stream.

    The tile legalizer pairs one LDWEIGHTS with EVERY matmul, so the two
    512-col matmuls that share a stationary tile reload it back-to-back
    (~113ns each, fully serialized on the PE queue -- ~16us over the
    kernel).  An LDW is redundant when the weights signature (access
    pattern + offset + tile_position) matches the live weights in every
    array row-strip it covers and it carries no materialized semaphore
    wait.  Nothing references LDWs as a dependency (verified), so removal
    is a pure list edit."""
    from concourse.mybir import InstLdweights, InstMatmult

    for f in nc.m.functions:
        for bb in f.blocks:
            insts = list(bb.instructions)
            live = []  # list of (row_lo, row_hi, sig)
            keep = []
            removed = 0
            for i in insts:
                if isinstance(i, InstLdweights):
                    ap = i.ins[0]
                    tp = i.tile_position or (0, 0)
                    nrows = ap.ap[0][1]
                    lo, hi = tp[0], tp[0] + nrows
                    sig = (str(ap.ap), ap.offset, str(ap.dtype), tp,
                           str(i.perf_mode), str(i.is_transpose))
                    cover = [e for e in live if not (e[1] <= lo or e[0] >= hi)]
                    same = (len(cover) == 1 and cover[0][0] == lo
                            and cover[0][1] == hi and cover[0][2] == sig)
                    has_wait = (i.sync_info is not None
                                and len(i.sync_info.on_wait) > 0)
                    if same and not has_wait:
                        removed += 1
                        continue
                    live = [e for e in live if (e[1] <= lo or e[0] >= hi)]
                    live.append((lo, hi, sig))
                keep.append(i)
            if removed:
                del bb.instructions[:]
                for i in keep:
                    bb.instructions.append(i)

B = 8
S = 1024
UNITS = 1024
H = 16
DH = 64
N_CORES = 8
KT = S // 128  # max key tiles per batch

BF16 = mybir.dt.bfloat16
F32 = mybir.dt.float32


def _build_nc(kbs):
    """Build the SPMD Bass program. kbs: per-batch number of 128-key tiles."""
    nc = bacc.Bacc("TRN2", target_bir_lowering=False, debug=False,
                   num_devices=N_CORES)
    qT = nc.dram_tensor("qt", [B, 128, S], BF16, kind="ExternalInput").ap()
    kT = nc.dram_tensor("kt", [B, 128, S], BF16, kind="ExternalInput").ap()
    vE = nc.dram_tensor("vt", [B, 128, KT, 256], BF16, kind="ExternalInput").ap()
    # unnormalized outputs: per (b, head) [65, S]; row 64 = denominator.
    # Shipped bf16 (halves the output DMA); the host divides in f32.
    o = nc.dram_tensor("o", [B, 2, 65, S], BF16, kind="ExternalOutput").ap()

    cb = nc.const_aps.aps[(BF16, 1.0)]  # [128, 1] framework const

    with tile.TileContext(nc) as tc:
        with (
            tc.tile_pool(name="qk", bufs=2) as qk_pool,
            tc.tile_pool(name="v", bufs=2) as v_pool,
            tc.tile_pool(name="w", bufs=32) as w_pool,
            tc.tile_pool(name="ot", bufs=4) as o_pool,
            tc.tile_pool(name="sc", bufs=2, space="PSUM") as sc_pool,
            tc.tile_pool(name="acc", bufs=2, space="PSUM") as acc_pool,
        ):
            def junk(sc_t, n, width=256):
                """n PE warm-keeper matmuls with zero PSUM footprint: const
                [128,1] bf16 broadcast to a full 128x128 stationary tile and
                a [128,width] moving tile (the HAM activity monitor only
                counts real array occupancy), written into a score tile that
                the next QK overwrites."""
                lhsT = bass.AP(tensor=cb.tensor, offset=cb.offset,
                               ap=[cb.ap[0], [0, 128]])
                rhs = bass.AP(tensor=cb.tensor, offset=cb.offset,
                              ap=[cb.ap[0], [0, width]])
                for _ in range(n):
                    nc.tensor.matmul(sc_t[:, 0:width], lhsT=lhsT,
                                     rhs=rhs, start=True, stop=True,
                                     skip_group_check=True)

            # ACT exp table preload (~2.7us) while the preamble/DMAs fly.
            wexp = qk_pool.tile([1, 8], F32, tag="wexp", name="wexp", bufs=1)
            nc.gpsimd.memset(wexp[:], 0.0)
            nc.scalar.activation(wexp[:], wexp[:],
                                 mybir.ActivationFunctionType.Exp)

            # Load every batch's inputs up front (fits easily in SBUF) so no
            # QK phase ever waits on DMA. First batch's K/Q head-A halves go
            # first so the first QK can start ~1us earlier; V tiles are only
            # needed one batch later.
            # First batch small, then largest-first, smallest last (short
            # tail after the final exp).
            srt = sorted(range(B), key=lambda i: -kbs[i])
            order = [srt[-2]] + srt[:-2] + [srt[-1]]
            b0 = order[0]
            qts, kts, vts = {}, {}, {}
            for b in order:
                qts[b] = qk_pool.tile([128, S], BF16, tag=f"qt{b}",
                                      name=f"qt{b}", bufs=1)
                # K only needs the valid key columns
                kts[b] = qk_pool.tile([128, 128 * kbs[b]], BF16, tag=f"kt{b}",
                                      name=f"kt{b}", bufs=1)
            for b in order:
                vts[b] = v_pool.tile([128, kbs[b], 256], BF16, tag=f"vt{b}",
                                     name=f"vt{b}", bufs=1)
            kb0 = kbs[b0]
            nc.sync.dma_start(out=kts[b0][0:64, :], in_=kT[b0, 0:64, :128 * kb0])
            nc.sync.dma_start(out=qts[b0][0:64, :], in_=qT[b0, 0:64, :])
            nc.sync.dma_start(out=kts[b0][64:128, :],
                              in_=kT[b0, 64:128, :128 * kb0])
            nc.sync.dma_start(out=qts[b0][64:128, :], in_=qT[b0, 64:128, :])
            # Interleave: batch b+1's K/Q, then batch b's V — each batch's V
            # arrives before its A-V drip starts, K/Q before its QK phase.
            for i, b in enumerate(order[1:]):
                nc.sync.dma_start(out=kts[b][:], in_=kT[b, :, :128 * kbs[b]])
                nc.sync.dma_start(out=qts[b][:], in_=qT[b])
                prev = order[i]
                nc.sync.dma_start(out=vts[prev][:],
                                  in_=vE[prev, :, :kbs[prev], :])
            bl = order[-1]
            nc.sync.dma_start(out=vts[bl][:], in_=vE[bl, :, :kbs[bl], :])

            # Flat substep stream: one substep per (batch, key-tile, head).
            # Normal batches tile-major (t,h); the last batch head-major so
            # head A's A-V (which needs every A exp) overlaps head B's exp
            # phase instead of extending the kernel tail.
            recs = {}
            subs = []
            for bi, b in enumerate(order):
                kb = kbs[b]
                last = bi == len(order) - 1
                recs[b] = {"b": b, "kb": kb, "wts": [[None] * kb, [None] * kb],
                           "vt": vts[b], "last": last}
                if last:
                    sl = [(t, h) for h in range(2) for t in range(kb)]
                else:
                    sl = [(t, h) for t in range(kb) for h in range(2)]
                subs.extend((b, t, h) for (t, h) in sl)

            def emit_qk(s, warm=2, warm_width=256, early=False):
                b, t, h = s
                base = 64 * h
                if early:
                    # The acc PSUM banks sit idle until the first batch's
                    # exps complete; borrowing them as extra score slots
                    # deepens the QK pipeline to 4 at the start so the PE
                    # runs real QKs back-to-back through the HAM activity
                    # window (reliable 2.4GHz flip on every core) instead
                    # of idling exp-paced at 1.2GHz.
                    sc = acc_pool.tile([128, S], F32, tag="acc", name="esc")
                else:
                    sc = sc_pool.tile([128, S], F32, tag="sc", name="sc")
                junk(sc, warm, warm_width)
                for qc in range(2):
                    nc.tensor.matmul(
                        sc[:, qc * 512:(qc + 1) * 512],
                        lhsT=kts[b][base:base + 64, t * 128:(t + 1) * 128],
                        rhs=qts[b][base:base + 64, qc * 512:(qc + 1) * 512],
                        start=True, stop=True,
                    )
                return sc

            # Schraudolph constants for the DVE exp approximation:
            # bf16 bits of exp(s) ~= int16(A*s + B); A = 128/ln(2), B centers
            # the (1+f)/2^f mantissa error (max ~4% on weights, ~1% on the
            # softmax-averaged output). ScalarE alone is the exp bottleneck
            # (~80us); shifting every 3rd tile to the otherwise-idle DVE
            # takes ~20us off the critical path.
            EXP_A = 184.6649652337873
            EXP_B = 16250.5

            def emit_exp(s, sc, on_dve):
                b, t, h = s
                wt = w_pool.tile([128, S], BF16, tag="w", name=f"w{b}_{t}_{h}")
                if on_dve:
                    nc.vector.tensor_scalar(
                        wt[:].bitcast(mybir.dt.int16), sc[:],
                        EXP_A, EXP_B,
                        mybir.AluOpType.mult, mybir.AluOpType.add)
                else:
                    nc.scalar.activation(wt[:], sc[:],
                                         mybir.ActivationFunctionType.Exp)
                recs[b]["wts"][h][t] = wt

            def emit_av(p, h, t):
                """One A-V accumulation step: outT[65, S] += V_t.T @ W_t^T
                (two 512-col matmuls, one per PSUM bank half). Stationary is
                the 65 used V columns only (LDWEIGHTS cost scales with
                stationary columns: 65 -> ~54ns vs 128 -> ~107ns)."""
                if t == 0:
                    p["acc"] = acc_pool.tile([128, S], F32, tag="acc",
                                             name=f"acc{p['b']}_{h}")
                kb = p["kb"]
                for half in range(2):
                    nc.tensor.matmul(
                        p["acc"][0:65, half * 512:(half + 1) * 512],
                        lhsT=p["vt"][:, t, h * 128:h * 128 + 65],
                        rhs=p["wts"][h][t][:, half * 512:(half + 1) * 512],
                        start=(t == 0), stop=(t == kb - 1),
                    )
                if t == kb - 1:
                    epilogue(p, h)

            epi_no = [0]

            def epilogue(p, h):
                """Ship head h's unnormalized sums: PSUM -> SBUF(bf16) -> HBM.
                Copied and DMA'd in halves so the first DMA overlaps the
                second copy. Heads alternate between ScalarE and VectorE so
                neither exp engine eats the whole ~19us of copy work."""
                ot = o_pool.tile([65, 2, 512], BF16, tag="ot", name="ot")
                ov = o[p["b"], h]
                on_act = epi_no[0] % 2 == 0
                epi_no[0] += 1
                for c in range(2):
                    src = p["acc"][0:65, c * 512:(c + 1) * 512]
                    if on_act:
                        nc.scalar.copy(ot[:, c, :], src)
                    else:
                        nc.vector.tensor_scalar_mul(ot[:, c, :], src, 1.0)
                    nc.sync.dma_start(out=ov[:, c * 512:(c + 1) * 512],
                                      in_=ot[:, c, :])

            # Global stream. exp(s) is emitted together with QK(s+1): with
            # the 3-deep score pool, QK(s+1) reuses a slot freed 1.5 tiles
            # ago, so the PE runs ahead and ScalarE never waits. A-V steps
            # of finished batches drip out at a bounded per-window rate so
            # they never pile up in front of the next QK pair.
            avq = []  # (batch record, head, t) FIFO
            total_steps = sum(kbs)
            kb0 = kbs[order[0]]
            # Exps on different engines run in parallel (independent score/
            # weight tiles), so strict ACT/DVE alternation doubles the exp
            # throughput. Use it where the stream is exp-paced (the first
            # two batches, before any A-V work exists, and the last batch's
            # tail); elsewhere the PE paces and a 2:1 split keeps DVE free
            # for the epilogue copies.
            nfirst2 = 2 * (kbs[order[0]] + kbs[order[1]])
            nlast = 2 * kbs[order[-1]]
            def dve_pick(i):
                if i < nfirst2 or i >= len(subs) - nlast:
                    return i % 2 == 1
                return i % 7 in (1, 3, 5)
            step_no = 0
            first_sc = emit_qk(subs[0], warm=4, warm_width=384)
            pending_sc = first_sc
            for i, s in enumerate(subs):
                b, t, h = s
                rec = recs[b]
                kb, last = rec["kb"], rec["last"]
                emit_exp(s, pending_sc, on_dve=dve_pick(i))
                if i + 1 < len(subs):
                    # substeps 1-2 borrow the idle acc banks as score slots
                    # (4-deep QK pipeline keeps the PE dense through the HAM
                    # window); light junk keeps density up through the rest
                    # of the first batch's exp-paced phase.
                    pending_sc = emit_qk(subs[i + 1],
                                         warm=2 if i < 2 * kb0 else 0,
                                         warm_width=256,
                                         early=i < 2)
                if last and h == 0 and t == kb - 1:
                    # head A complete: its A-V can interleave from here
                    avq.extend((rec, 0, tt) for tt in range(kb))
                step_no += 0 if h else 1
                rem = max(1, total_steps - step_no)
                rate = -(-len(avq) // min(rem, 8))
                cap = 6 if rem <= 4 else 3
                if h == 1 or last:
                    for _ in range(min(rate, cap)):
                        if avq:
                            emit_av(*avq.pop(0))
                if h == 1 and t == kb - 1:
                    if last:
                        avq.extend((rec, 1, tt) for tt in range(kb))
                    else:
                        avq.extend((rec, hh, tt)
                                   for hh in range(2) for tt in range(kb))

            while avq:
                emit_av(*avq.pop(0))
    _dedup_ldweights(nc)
    nc.compile()
    return nc


_NC_CACHE = {}


def _get_nc(kbs):
    key = tuple(kbs)
    if key not in _NC_CACHE:
        _NC_CACHE[key] = _build_nc(key)
    return _NC_CACHE[key]


def kernel(memory, query, b, seq_len):
    memory = np.asarray(memory)
    query = np.asarray(query)
    bias = np.asarray(b, dtype=np.float32)
    seq_len = np.asarray(seq_len).reshape(-1).astype(np.int64)

    sl = seq_len.copy()
    kbs = [int(min(KT, max(1, -(-int(s) // 128)))) if s > 0 else KT for s in sl]

    # emb[b, k] = exp(bias[k]) * valid; fully-masked batch -> plain softmax
    pos = np.arange(S)[None, :]
    valid = (pos < sl[:, None]) | (sl[:, None] == 0)
    emb = np.exp(bias)[None, :] * valid.astype(np.float32)  # [B, S]

    qh = (query.astype(np.float32) * (DH ** -0.5)).reshape(B, S, H, DH)
    kh = memory[:, :, :UNITS].astype(np.float32).reshape(B, S, H, DH)
    vh = memory[:, :, UNITS:].astype(np.float32).reshape(B, S, H, DH)
    vh = vh * emb[:, :, None, None]  # [B, S, H, DH] value rows pre-masked

    bf = ml_dtypes.bfloat16
    # [B, S, H, DH] -> [B, H, DH, S] transposed layouts
    qTfull = np.ascontiguousarray(qh.transpose(0, 2, 3, 1)).astype(bf)
    kTfull = np.ascontiguousarray(kh.transpose(0, 2, 3, 1)).astype(bf)
    # [B, S, H, DH] -> [B, (t p), H, DH] -> [B, 128, KT, H, DH]
    vtiles = np.ascontiguousarray(
        vh.reshape(B, KT, 128, H, DH).transpose(0, 2, 1, 3, 4)).astype(bf)
    embt = np.ascontiguousarray(
        emb.reshape(B, KT, 128).transpose(0, 2, 1)).astype(bf)  # [B, 128, KT]

    in_maps = []
    for c in range(N_CORES):
        hA, hB = 2 * c, 2 * c + 1
        qTc = np.concatenate([qTfull[:, hA], qTfull[:, hB]], axis=1)
        kTc = np.concatenate([kTfull[:, hA], kTfull[:, hB]], axis=1)
        vEc = np.zeros((B, 128, KT, 256), dtype=bf)
        vEc[..., 0:64] = vtiles[:, :, :, hA, :]
        vEc[..., 64] = embt
        vEc[..., 128:192] = vtiles[:, :, :, hB, :]
        vEc[..., 192] = embt
        in_maps.append({
            "qt": np.ascontiguousarray(qTc),
            "kt": np.ascontiguousarray(kTc),
            "vt": np.ascontiguousarray(vEc),
        })

    nc = _get_nc(kbs)
    res = run_bass_kernel_spmd(nc, in_maps, core_ids=list(range(N_CORES)))

    out = np.empty((B, S, UNITS), dtype=np.float32)
    for c in range(N_CORES):
        oc = np.asarray(res.results[c]["o"], dtype=np.float32)  # [B,2,65,S]
        num = oc[:, :, 0:64, :]                      # [B, 2, 64, S]
        den = oc[:, :, 64:65, :]                     # [B, 2, 1, S]
        core = (num / den).transpose(0, 3, 1, 2)     # [B, S, 2, 64]
        out[:, :, 128 * c:128 * (c + 1)] = core.reshape(B, S, 128)
    return out



# revision 68
# speedup vs baseline: 1.0742x; 1.0563x over previous
"""Sparse-attention Trainium2 kernel (nn_Attention_81398220193933).

Strategy (tensor-parallel over heads, 2 heads per NeuronCore):
  - Host pre-lays-out per-core tensors:
      qT  [B, 128, S]  bf16 : rows 0:64 = headA Q^T / sqrt(dh), rows 64:128 = headB
      kT  [B, 128, S]  bf16 : same for K^T (only the valid kb*128 key
           columns are DMA'd)
      vE  [B, 128, 8, 256] bf16 : per k-tile t, partition p = key t*128+p,
           cols [0:64]=V_A*emb, [64]=emb, [128:192]=V_B*emb, [192]=emb
           where emb[b,k] = exp(bias[k]) * (k < seq_len[b]) (all-valid if
           seq_len==0).
    Folding the additive key bias + mask multiplicatively into V makes the
    softmax mask/bias free on-device and lets fully-masked k-tiles be
    skipped. Softmax max-subtraction is unnecessary: logits are O(+-6).
  - Device, one PAIR stream iteration per (batch b, key-tile t):
      QK pair: scores^T [128, 1024] per head; head A's K stationary sits in
        array rows 0:64 (tile_position (0,0)), head B's in rows 64:128
        ((64,0), auto-derived from the SBUF base partition). Emitted
        interleaved B0,A0,B1,A1 so the two heads' matmuls run CONCURRENTLY
        in the two row-strips (~600ns instead of ~1070ns per pair); head B
        goes first because its exp rides the slower DVE leg.
      exp pair: head A on ScalarE (true exp, ~1114ns), head B on VectorE
        (Schraudolph bf16-bit affine: int16(A*s+B), ~1224ns) in parallel.
      A-V: outT[65, q] += V_tile(+emb col).T @ W^T accumulated over t in
        PSUM per head; row 64 accumulates the softmax denominator (the
        2-head A-V cannot be packed into one 128-col stationary: 65+65>128).
        V is zero-padded to 128 stationary columns (cheap HAM warmth).
      Epilogue per (batch, head): [65,512] PSUM -> SBUF bf16 copies, one
        chunk per exp-engine per pair iteration (each rides the engine's
        natural idle window while the PE runs the next QK; two chunks on
        one engine would delay the next exp by ~700ns and stretch the whole
        pipeline cycle). Then DMA to HBM; the num/den division happens on
        the host in f32.
  - Pipeline: PSUM = 2 score slots (4 banks) + 2 A-V accumulators (4
    banks). The steady-state cycle exp(i) -> QK(i+1) -> exp(i+1) runs at
    ~1.85us/pair; A-V matmuls drip with a 1-pair lag to fill the PE while
    exps run. A 3rd score slot would break the exp->QK serialization but
    PSUM is full (verified: every acc-shrinking scheme loses more).
  - _dedup_ldweights removes the legalizer's redundant per-matmul
    LDWEIGHTS (same stationary, same row-strip) -- ~16us of serialized PE
    time otherwise.
  - HAM clock-gate (binary 1.2/2.4GHz, ~3.4us windows): a 10x512 junk-
    matmul burst in front of pair 0 flips every core to 2.4GHz by ~11-13us
    reliably; weaker ramps measured faster on lucky cores but worse
    max-core (the harness metric). Mid-stream fillers for the ~300ns/pair
    idle were all net-negative; the occasional mid-kernel re-throttle is
    cheaper.
"""

import numpy as np
import ml_dtypes

import concourse.bass as bass
import concourse.mybir as mybir
import concourse.tile as tile
from concourse import bacc
from concourse.bass_utils import run_bass_kernel_spmd


def _dedup_ldweights(nc):
    """Remove redundant InstLdweights from the PE stream.

    The tile legalizer pairs one LDWEIGHTS with EVERY matmul, so the two
    512-col matmuls that share a stationary tile reload it back-to-back
    (~113ns each, fully serialized on the PE queue -- ~16us over the
    kernel).  An LDW is redundant when the weights signature (access
    pattern + offset + tile_position) matches the live weights in every
    array row-strip it covers and it carries no materialized semaphore
    wait.  Nothing references LDWs as a dependency (verified), so removal
    is a pure list edit."""
    from concourse.mybir import InstLdweights, InstMatmult

    for f in nc.m.functions:
        for bb in f.blocks:
            insts = list(bb.instructions)
            live = []  # list of (row_lo, row_hi, sig)
            keep = []
            removed = 0
            for i in insts:
                if isinstance(i, InstLdweights):
                    ap = i.ins[0]
                    tp = i.tile_position or (0, 0)
                    nrows = ap.ap[0][1]
                    lo, hi = tp[0], tp[0] + nrows
                    sig = (str(ap.ap), ap.offset, str(ap.dtype), tp,
                           str(i.perf_mode), str(i.is_transpose))
                    cover = [e for e in live if not (e[1] <= lo or e[0] >= hi)]
                    same = (len(cover) == 1 and cover[0][0] == lo
                            and cover[0][1] == hi and cover[0][2] == sig)
                    has_wait = (i.sync_info is not None
                                and len(i.sync_info.on_wait) > 0)
                    if same and not has_wait:
                        removed += 1
                        continue
                    live = [e for e in live if (e[1] <= lo or e[0] >= hi)]
                    live.append((lo, hi, sig))
                keep.append(i)
            if removed:
                del bb.instructions[:]
                for i in keep:
                    bb.instructions.append(i)

B = 8
S = 1024
UNITS = 1024
H = 16
DH = 64
N_CORES = 8
KT = S // 128  # max key tiles per batch

BF16 = mybir.dt.bfloat16
F32 = mybir.dt.float32


def _build_nc(kbs):
    """Build the SPMD Bass program. kbs: per-batch number of 128-key tiles."""
    nc = bacc.Bacc("TRN2", target_bir_lowering=False, debug=False,
                   num_devices=N_CORES)
    qT = nc.dram_tensor("qt", [B, 128, S], BF16, kind="ExternalInput").ap()
    kT = nc.dram_tensor("kt", [B, 128, S], BF16, kind="ExternalInput").ap()
    vE = nc.dram_tensor("vt", [B, 128, KT, 256], BF16, kind="ExternalInput").ap()
    # unnormalized outputs: per (b, head) [65, S]; row 64 = denominator.
    # Shipped bf16 (halves the output DMA); the host divides in f32.
    o = nc.dram_tensor("o", [B, 2, 65, S], BF16, kind="ExternalOutput").ap()

    cb = nc.const_aps.aps[(BF16, 1.0)]  # [128, 1] framework const

    with tile.TileContext(nc) as tc:
        with (
            tc.tile_pool(name="qk", bufs=2) as qk_pool,
            tc.tile_pool(name="v", bufs=2) as v_pool,
            tc.tile_pool(name="w", bufs=32) as w_pool,
            tc.tile_pool(name="ot", bufs=4) as o_pool,
            tc.tile_pool(name="sc", bufs=2, space="PSUM") as sc_pool,
            tc.tile_pool(name="acc", bufs=2, space="PSUM") as acc_pool,
        ):
            def junk(sc_t, n, width=256):
                """n PE warm-keeper matmuls with zero PSUM footprint: const
                [128,1] bf16 broadcast to a full 128x128 stationary tile and
                a [128,width] moving tile (the HAM activity monitor only
                counts real array occupancy), written into a score tile that
                the next QK overwrites."""
                lhsT = bass.AP(tensor=cb.tensor, offset=cb.offset,
                               ap=[cb.ap[0], [0, 128]])
                rhs = bass.AP(tensor=cb.tensor, offset=cb.offset,
                              ap=[cb.ap[0], [0, width]])
                for _ in range(n):
                    nc.tensor.matmul(sc_t[:, 0:width], lhsT=lhsT,
                                     rhs=rhs, start=True, stop=True,
                                     skip_group_check=True)

            # ACT exp table preload (~2.7us) while the preamble/DMAs fly.
            wexp = qk_pool.tile([1, 8], F32, tag="wexp", name="wexp", bufs=1)
            nc.gpsimd.memset(wexp[:], 0.0)
            nc.scalar.activation(wexp[:], wexp[:],
                                 mybir.ActivationFunctionType.Exp)

            # Load every batch's inputs up front (fits easily in SBUF) so no
            # QK phase ever waits on DMA. First batch's K/Q head-A halves go
            # first so the first QK can start ~1us earlier; V tiles are only
            # needed one batch later.
            # First batch small, then largest-first, smallest last (short
            # tail after the final exp).
            srt = sorted(range(B), key=lambda i: -kbs[i])
            order = [srt[-2]] + srt[:-2] + [srt[-1]]
            b0 = order[0]
            qts, kts, vts = {}, {}, {}
            for b in order:
                qts[b] = qk_pool.tile([128, S], BF16, tag=f"qt{b}",
                                      name=f"qt{b}", bufs=1)
                # K only needs the valid key columns
                kts[b] = qk_pool.tile([128, 128 * kbs[b]], BF16, tag=f"kt{b}",
                                      name=f"kt{b}", bufs=1)
            for b in order:
                vts[b] = v_pool.tile([128, kbs[b], 256], BF16, tag=f"vt{b}",
                                     name=f"vt{b}", bufs=1)
            kb0 = kbs[b0]
            nc.sync.dma_start(out=kts[b0][0:64, :], in_=kT[b0, 0:64, :128 * kb0])
            nc.sync.dma_start(out=qts[b0][0:64, :], in_=qT[b0, 0:64, :])
            nc.sync.dma_start(out=kts[b0][64:128, :],
                              in_=kT[b0, 64:128, :128 * kb0])
            nc.sync.dma_start(out=qts[b0][64:128, :], in_=qT[b0, 64:128, :])
            # Interleave: batch b+1's K/Q, then batch b's V — each batch's V
            # arrives before its A-V drip starts, K/Q before its QK phase.
            # (Issues serialize ~1.4us apiece on the Sync queue; measured
            # attempts to reorder V earlier or split issues onto the GpSimd
            # SWDGE queue both regressed 12-17us -- keep this exact order.)
            for i, b in enumerate(order[1:]):
                nc.sync.dma_start(out=kts[b][:], in_=kT[b, :, :128 * kbs[b]])
                nc.sync.dma_start(out=qts[b][:], in_=qT[b])
                prev = order[i]
                nc.sync.dma_start(out=vts[prev][:],
                                  in_=vE[prev, :, :kbs[prev], :])
            bl = order[-1]
            nc.sync.dma_start(out=vts[bl][:], in_=vE[bl, :, :kbs[bl], :])

            # Pair stream: one iteration per (batch, key-tile). The two
            # heads' QKs are emitted adjacently as B0,A0,B1,A1: head A's
            # stationary lives in array rows 0:64 (tile_position (0,0)) and
            # head B's in rows 64:128 ((64,0), auto-derived from the SBUF
            # base partition), so consecutive matmuls land in different
            # row-strips and run CONCURRENTLY (observed dstart ~5ns),
            # nearly halving QK's PE occupancy.
            recs = {}
            pairs = []
            for bi, b in enumerate(order):
                kb = kbs[b]
                recs[b] = {"b": b, "kb": kb, "wts": [[None] * kb, [None] * kb],
                           "vt": vts[b]}
                pairs.extend((b, t) for t in range(kb))

            def emit_qk_pair(p, warm=0, warm_width=256, early=False):
                b, t = p
                if early:
                    # The acc PSUM banks sit idle until the first batch's
                    # A-V starts; borrowing them as a second pair-slot
                    # doubles the QK pipeline depth at the start so the PE
                    # runs real QKs back-to-back through the HAM activity
                    # window (2.4GHz flip) instead of idling exp-paced.
                    sc_a = acc_pool.tile([128, S], F32, tag="acc", name="esc")
                    sc_b = acc_pool.tile([128, S], F32, tag="acc", name="esc")
                else:
                    sc_a = sc_pool.tile([128, S], F32, tag="sc", name="sc")
                    sc_b = sc_pool.tile([128, S], F32, tag="sc", name="sc")
                junk(sc_a, warm, warm_width)
                # Head B first: its exp runs on the slower DVE leg
                # (Schraudolph 1224ns vs ACT 1114ns), so giving its QK
                # chunks the early strip slots balances the two
                # exp->QK->exp pipeline legs.
                for qc in range(2):
                    for h, sc in ((1, sc_b), (0, sc_a)):
                        base = 64 * h
                        nc.tensor.matmul(
                            sc[:, qc * 512:(qc + 1) * 512],
                            lhsT=kts[b][base:base + 64, t * 128:(t + 1) * 128],
                            rhs=qts[b][base:base + 64, qc * 512:(qc + 1) * 512],
                            start=True, stop=True,
                        )
                return (sc_a, sc_b)

            # Schraudolph constants for the DVE exp approximation:
            # bf16 bits of exp(s) ~= int16(A*s + B); A = 128/ln(2), B centers
            # the (1+f)/2^f mantissa error (max ~4% on weights, ~1% on the
            # softmax-averaged output). Head A's exp always runs on ScalarE
            # (true exp) and head B's on VectorE (Schraudolph) -- the two
            # run in parallel, one pair per ~1.25us.
            EXP_A = 184.6649652337873
            EXP_B = 16250.5

            def emit_exp_pair(p, scs):
                b, t = p
                for h in (0, 1):
                    wt = w_pool.tile([128, S], BF16, tag="w",
                                     name=f"w{b}_{t}_{h}")
                    if h == 1:
                        nc.vector.tensor_scalar(
                            wt[:].bitcast(mybir.dt.int16), scs[h][:],
                            EXP_A, EXP_B,
                            mybir.AluOpType.mult, mybir.AluOpType.add)
                    else:
                        nc.scalar.activation(wt[:], scs[h][:],
                                             mybir.ActivationFunctionType.Exp)
                    recs[b]["wts"][h][t] = wt

            def emit_av(p, h, t):
                """One A-V accumulation step: outT[65, S] += V_t.T @ W_t^T
                (two 512-col matmuls, one per PSUM bank half). The V tile is
                zero-padded to 128 stationary columns: the wider LDWEIGHTS
                (+53ns) is free PE-busy filler during the exp-gated stall and
                keeps the HAM idle fraction below the re-throttle threshold;
                output rows 65:128 are zeros and never read."""
                if t == 0:
                    p[f"acc{h}"] = acc_pool.tile([128, S], F32, tag="acc",
                                                 name=f"acc{p['b']}_{h}")
                kb = p["kb"]
                for half in range(2):
                    nc.tensor.matmul(
                        p[f"acc{h}"][0:65, half * 512:(half + 1) * 512],
                        lhsT=p["vt"][:, t, h * 128:h * 128 + 65],
                        rhs=p["wts"][h][t][:, half * 512:(half + 1) * 512],
                        start=(t == 0), stop=(t == kb - 1),
                    )
                if t == kb - 1:
                    epiq.append((p, h, 0, cur_pair[0]))
                    epiq.append((p, h, 1, cur_pair[0]))

            epiq = []  # deferred epilogue chunks: (rec, head, chunk, pair)
            epi_no = [0]
            cur_pair = [0]

            def emit_epi_chunk(min_age=2, on_act=None):
                """Ship one [65,512] chunk of a finished head's unnormalized
                sums: PSUM -> SBUF(bf16) -> HBM. At most one chunk per
                engine per pair iteration: a ~680ns copy fits in each exp
                engine's idle window (exp(i) can't start until QK(i) is
                done), so the copies ride for free instead of delaying the
                next exp -- emitting two on ONE engine measured +1.3us on
                that pair's cycle. A chunk is held until its A-V chain-stop
                matmul is at least 2 pairs old, else the strict-FIFO exp
                engine blocks on the not-yet-run accumulation."""
                if not epiq or epiq[0][3] > cur_pair[0] - min_age:
                    return
                p, h, c, _ = epiq.pop(0)
                if c == 0:
                    p[f"ot{h}"] = o_pool.tile([65, 2, 512], BF16, tag="ot",
                                              name="ot")
                ot = p[f"ot{h}"]
                src = p[f"acc{h}"][0:65, c * 512:(c + 1) * 512]
                if on_act is None:
                    on_act = epi_no[0] % 8 < 5
                epi_no[0] += 1
                if on_act:
                    nc.scalar.copy(ot[:, c, :], src)
                else:
                    nc.vector.tensor_scalar_mul(ot[:, c, :], src, 1.0)
                nc.sync.dma_start(out=o[p["b"], h][:, c * 512:(c + 1) * 512],
                                  in_=ot[:, c, :])

            # Global stream, per pair i:
            #   epi chunk(s) -> exps(i) -> A-V drip (pairs <= i-lag) ->
            #   QK(i+1)
            # The drip sits BEFORE the next QK in the PE queue so the PE
            # chews A-V work while exps(i) run; QK(i+1) then starts as soon
            # as its score slots free (exp(i-1) with the 2-pair pipeline).
            # A-V for pair i only needs exp(i), so a lag of 1 pair is
            # enough in steady state; the first batch uses lag 2 because
            # the exps are still table-load/cold-clock delayed there.
            avq = []  # (batch record, head, t, pair idx) FIFO
            kb0 = kbs[order[0]]
            pend = emit_qk_pair(pairs[0], warm=10, warm_width=512)
            for i, p in enumerate(pairs):
                b, t = p
                rec = recs[b]
                cur_pair[0] = i
                if i < len(pairs) - 2:
                    # no epi copies in the last two pairs: they inflate the
                    # monotonic ACT/DVE counters that gate the final A-V
                    # matmuls (+0.6us on the tail)
                    emit_epi_chunk(on_act=True)
                    if len(epiq) >= 3 or i >= len(pairs) - 8:
                        # second chunk rides the DVE's window: drains the
                        # 4-chunk batch-boundary backlog in 2 pairs (the
                        # next batch's A-V chain waits on its acc banks'
                        # last epi read) and keeps the post-loop tail short
                        emit_epi_chunk(on_act=False)
                emit_exp_pair(p, pend)
                # Ramp notes: batch 0's AVs wait its V-tile DMA (~16.6us,
                # serial Sync issues) and block the FIFO ahead of batch 1's
                # first QK, pushing exp(pair 3) to ~20.5us. Every attempted
                # fix measured WORSE on max-core: deferring the AVs via
                # lag 4 (98.2us), QK-before-drip ordering in the ramp
                # (100.9us), DMA reorders (111-116us). The drip-first
                # lag-2 schedule below is the validated optimum.
                lag = 2 if i < kb0 else 1
                while avq and avq[0][3] <= i - lag:
                    emit_av(*avq.pop(0)[:3])
                if i + 1 < len(pairs):
                    pend = emit_qk_pair(pairs[i + 1],
                                        warm=2 if i < kb0 else 0,
                                        warm_width=256,
                                        early=i == 0)
                avq.append((rec, 0, t, i))
                avq.append((rec, 1, t, i))

            while avq:
                emit_av(*avq.pop(0)[:3])
            j = 0
            while epiq:
                emit_epi_chunk(min_age=-10**9, on_act=j % 2 == 0)
                j += 1
    _dedup_ldweights(nc)
    nc.compile()
    return nc


_NC_CACHE = {}


def _get_nc(kbs):
    key = tuple(kbs)
    if key not in _NC_CACHE:
        _NC_CACHE[key] = _build_nc(key)
    return _NC_CACHE[key]


def kernel(memory, query, b, seq_len):
    memory = np.asarray(memory)
    query = np.asarray(query)
    bias = np.asarray(b, dtype=np.float32)
    seq_len = np.asarray(seq_len).reshape(-1).astype(np.int64)

    sl = seq_len.copy()
    kbs = [int(min(KT, max(1, -(-int(s) // 128)))) if s > 0 else KT for s in sl]

    # emb[b, k] = exp(bias[k]) * valid; fully-masked batch -> plain softmax
    pos = np.arange(S)[None, :]
    valid = (pos < sl[:, None]) | (sl[:, None] == 0)
    emb = np.exp(bias)[None, :] * valid.astype(np.float32)  # [B, S]

    qh = (query.astype(np.float32) * (DH ** -0.5)).reshape(B, S, H, DH)
    kh = memory[:, :, :UNITS].astype(np.float32).reshape(B, S, H, DH)
    vh = memory[:, :, UNITS:].astype(np.float32).reshape(B, S, H, DH)
    vh = vh * emb[:, :, None, None]  # [B, S, H, DH] value rows pre-masked

    bf = ml_dtypes.bfloat16
    # [B, S, H, DH] -> [B, H, DH, S] transposed layouts
    qTfull = np.ascontiguousarray(qh.transpose(0, 2, 3, 1)).astype(bf)
    kTfull = np.ascontiguousarray(kh.transpose(0, 2, 3, 1)).astype(bf)
    # [B, S, H, DH] -> [B, (t p), H, DH] -> [B, 128, KT, H, DH]
    vtiles = np.ascontiguousarray(
        vh.reshape(B, KT, 128, H, DH).transpose(0, 2, 1, 3, 4)).astype(bf)
    embt = np.ascontiguousarray(
        emb.reshape(B, KT, 128).transpose(0, 2, 1)).astype(bf)  # [B, 128, KT]

    in_maps = []
    for c in range(N_CORES):
        hA, hB = 2 * c, 2 * c + 1
        qTc = np.concatenate([qTfull[:, hA], qTfull[:, hB]], axis=1)
        kTc = np.concatenate([kTfull[:, hA], kTfull[:, hB]], axis=1)
        vEc = np.zeros((B, 128, KT, 256), dtype=bf)
        vEc[..., 0:64] = vtiles[:, :, :, hA, :]
        vEc[..., 64] = embt
        vEc[..., 128:192] = vtiles[:, :, :, hB, :]
        vEc[..., 192] = embt
        in_maps.append({
            "qt": np.ascontiguousarray(qTc),
            "kt": np.ascontiguousarray(kTc),
            "vt": np.ascontiguousarray(vEc),
        })

    nc = _get_nc(kbs)
    res = run_bass_kernel_spmd(nc, in_maps, core_ids=list(range(N_CORES)))

    out = np.empty((B, S, UNITS), dtype=np.float32)
    for c in range(N_CORES):
        oc = np.asarray(res.results[c]["o"], dtype=np.float32)  # [B,2,65,S]
        num = oc[:, :, 0:64, :]                      # [B, 2, 64, S]
        den = oc[:, :, 64:65, :]                     # [B, 2, 1, S]
        core = (num / den).transpose(0, 3, 1, 2)     # [B, S, 2, 64]
        out[:, :, 128 * c:128 * (c + 1)] = core.reshape(B, S, 128)
    return out



# revision 72
# speedup vs baseline: 1.1008x; 1.0247x over previous
"""Sparse-attention Trainium2 kernel (nn_Attention_81398220193933).

Strategy (tensor-parallel over heads, 2 heads per NeuronCore):
  - Host pre-lays-out per-core tensors:
      qT  [B, 128, S]  bf16 : rows 0:64 = headA Q^T / sqrt(dh), rows 64:128 = headB
      kT  [B, 128, S]  bf16 : same for K^T (only the valid kb*128 key
           columns are DMA'd)
      vE  [B, 128, 8, 256] bf16 : per k-tile t, partition p = key t*128+p,
           cols [0:64]=V_A*emb, [64]=emb, [128:192]=V_B*emb, [192]=emb
           where emb[b,k] = exp(bias[k]) * (k < seq_len[b]) (all-valid if
           seq_len==0).
    Folding the additive key bias + mask multiplicatively into V makes the
    softmax mask/bias free on-device and lets fully-masked k-tiles be
    skipped. Softmax max-subtraction is unnecessary: logits are O(+-6).
  - Device, one PAIR stream iteration per (batch b, key-tile t):
      QK pair: scores^T [128, 1024] per head; head A's K stationary sits in
        array rows 0:64 (tile_position (0,0)), head B's in rows 64:128
        ((64,0), auto-derived from the SBUF base partition). Emitted
        interleaved B0,A0,B1,A1 so the two heads' matmuls run CONCURRENTLY
        in the two row-strips (~600ns instead of ~1070ns per pair); head B
        goes first because its exp rides the slower DVE leg.
      exp pair: head A on ScalarE (true exp, ~1114ns), head B on VectorE
        (Schraudolph bf16-bit affine: int16(A*s+B), ~1224ns) in parallel.
      A-V: outT[65, q] += V_tile(+emb col).T @ W^T accumulated over t in
        PSUM per head; row 64 accumulates the softmax denominator (the
        2-head A-V cannot be packed into one 128-col stationary: 65+65>128).
        V is zero-padded to 128 stationary columns (cheap HAM warmth).
      Epilogue per (batch, head): [65,512] PSUM -> SBUF bf16 copies, one
        chunk per exp-engine per pair iteration (each rides the engine's
        natural idle window while the PE runs the next QK; two chunks on
        one engine would delay the next exp by ~700ns and stretch the whole
        pipeline cycle). Then DMA to HBM; the num/den division happens on
        the host in f32.
  - Pipeline: PSUM = 2 score slots (4 banks) + 2 A-V accumulators (4
    banks). The steady-state cycle exp(i) -> QK(i+1) -> exp(i+1) runs at
    ~1.85us/pair; A-V matmuls drip with a 1-pair lag to fill the PE while
    exps run. A 3rd score slot would break the exp->QK serialization but
    PSUM is full (verified: every acc-shrinking scheme loses more).
  - _dedup_ldweights removes the legalizer's redundant per-matmul
    LDWEIGHTS (same stationary, same row-strip) -- ~16us of serialized PE
    time otherwise.
  - HAM clock-gate (binary 1.2/2.4GHz, ~3.4us windows): a 10x512 junk-
    matmul burst in front of pair 0 flips every core to 2.4GHz by ~11-13us
    reliably; weaker ramps measured faster on lucky cores but worse
    max-core (the harness metric). Mid-stream fillers for the ~300ns/pair
    idle were all net-negative; the occasional mid-kernel re-throttle is
    cheaper.
"""

import numpy as np
import ml_dtypes

import concourse.bass as bass
import concourse.mybir as mybir
import concourse.tile as tile
from concourse import bacc
from concourse.bass_utils import run_bass_kernel_spmd


def _dedup_ldweights(nc):
    """Remove redundant InstLdweights from the PE stream.

    The tile legalizer pairs one LDWEIGHTS with EVERY matmul, so the two
    512-col matmuls that share a stationary tile reload it back-to-back
    (~113ns each, fully serialized on the PE queue -- ~16us over the
    kernel).  An LDW is redundant when the weights signature (access
    pattern + offset + tile_position) matches the live weights in every
    array row-strip it covers and it carries no materialized semaphore
    wait.  Nothing references LDWs as a dependency (verified), so removal
    is a pure list edit."""
    from concourse.mybir import InstLdweights, InstMatmult

    for f in nc.m.functions:
        for bb in f.blocks:
            insts = list(bb.instructions)
            live = []  # list of (row_lo, row_hi, sig)
            keep = []
            removed = 0
            for i in insts:
                if isinstance(i, InstLdweights):
                    ap = i.ins[0]
                    tp = i.tile_position or (0, 0)
                    nrows = ap.ap[0][1]
                    lo, hi = tp[0], tp[0] + nrows
                    sig = (str(ap.ap), ap.offset, str(ap.dtype), tp,
                           str(i.perf_mode), str(i.is_transpose))
                    cover = [e for e in live if not (e[1] <= lo or e[0] >= hi)]
                    same = (len(cover) == 1 and cover[0][0] == lo
                            and cover[0][1] == hi and cover[0][2] == sig)
                    has_wait = (i.sync_info is not None
                                and len(i.sync_info.on_wait) > 0)
                    if same and not has_wait:
                        removed += 1
                        continue
                    live = [e for e in live if (e[1] <= lo or e[0] >= hi)]
                    live.append((lo, hi, sig))
                keep.append(i)
            if removed:
                del bb.instructions[:]
                for i in keep:
                    bb.instructions.append(i)

B = 8
S = 1024
UNITS = 1024
H = 16
DH = 64
N_CORES = 8
KT = S // 128  # max key tiles per batch

BF16 = mybir.dt.bfloat16
F32 = mybir.dt.float32


def _build_nc(kbs):
    """Build the SPMD Bass program. kbs: per-batch number of 128-key tiles."""
    nc = bacc.Bacc("TRN2", target_bir_lowering=False, debug=False,
                   num_devices=N_CORES)
    qT = nc.dram_tensor("qt", [B, 128, S], BF16, kind="ExternalInput").ap()
    kT = nc.dram_tensor("kt", [B, 128, S], BF16, kind="ExternalInput").ap()
    vE = nc.dram_tensor("vt", [B, 128, KT, 256], BF16, kind="ExternalInput").ap()
    # unnormalized outputs: per (b, head) [65, S]; row 64 = denominator.
    # Shipped bf16 (halves the output DMA); the host divides in f32.
    o = nc.dram_tensor("o", [B, 2, 65, S], BF16, kind="ExternalOutput").ap()

    cb = nc.const_aps.aps[(BF16, 1.0)]  # [128, 1] framework const

    with tile.TileContext(nc) as tc:
        with (
            tc.tile_pool(name="qk", bufs=2) as qk_pool,
            tc.tile_pool(name="v", bufs=2) as v_pool,
            tc.tile_pool(name="w", bufs=32) as w_pool,
            tc.tile_pool(name="ot", bufs=4) as o_pool,
            tc.tile_pool(name="sc", bufs=2, space="PSUM") as sc_pool,
            tc.tile_pool(name="acc", bufs=2, space="PSUM") as acc_pool,
        ):
            def junk(sc_t, n, width=256):
                """n PE warm-keeper matmuls with zero PSUM footprint: const
                [128,1] bf16 broadcast to a full 128x128 stationary tile and
                a [128,width] moving tile (the HAM activity monitor only
                counts real array occupancy), written into a score tile that
                the next QK overwrites."""
                lhsT = bass.AP(tensor=cb.tensor, offset=cb.offset,
                               ap=[cb.ap[0], [0, 128]])
                rhs = bass.AP(tensor=cb.tensor, offset=cb.offset,
                              ap=[cb.ap[0], [0, width]])
                for _ in range(n):
                    nc.tensor.matmul(sc_t[:, 0:width], lhsT=lhsT,
                                     rhs=rhs, start=True, stop=True,
                                     skip_group_check=True)

            # ACT exp table preload (~2.7us) while the preamble/DMAs fly.
            wexp = qk_pool.tile([1, 8], F32, tag="wexp", name="wexp", bufs=1)
            nc.gpsimd.memset(wexp[:], 0.0)
            nc.scalar.activation(wexp[:], wexp[:],
                                 mybir.ActivationFunctionType.Exp)

            # Load every batch's inputs up front (fits easily in SBUF) so no
            # QK phase ever waits on DMA. First batch's K/Q head-A halves go
            # first so the first QK can start ~1us earlier; V tiles are only
            # needed one batch later.
            # First batch small, then largest-first, smallest last (short
            # tail after the final exp).
            srt = sorted(range(B), key=lambda i: -kbs[i])
            order = [srt[-2]] + srt[:-2] + [srt[-1]]
            b0 = order[0]
            qts, kts, vts = {}, {}, {}
            for b in order:
                qts[b] = qk_pool.tile([128, S], BF16, tag=f"qt{b}",
                                      name=f"qt{b}", bufs=1)
                # K only needs the valid key columns
                kts[b] = qk_pool.tile([128, 128 * kbs[b]], BF16, tag=f"kt{b}",
                                      name=f"kt{b}", bufs=1)
            for b in order:
                vts[b] = v_pool.tile([128, kbs[b], 256], BF16, tag=f"vt{b}",
                                     name=f"vt{b}", bufs=1)
            kb0 = kbs[b0]
            nc.sync.dma_start(out=kts[b0][0:64, :], in_=kT[b0, 0:64, :128 * kb0])
            nc.sync.dma_start(out=qts[b0][0:64, :], in_=qT[b0, 0:64, :])
            nc.sync.dma_start(out=kts[b0][64:128, :],
                              in_=kT[b0, 64:128, :128 * kb0])
            nc.sync.dma_start(out=qts[b0][64:128, :], in_=qT[b0, 64:128, :])
            # Interleave: batch b+1's K/Q, then batch b's V — each batch's V
            # arrives before its A-V drip starts, K/Q before its QK phase.
            # (Issues serialize ~1.4us apiece on the Sync queue; measured
            # attempts to reorder V earlier or split issues onto the GpSimd
            # SWDGE queue both regressed 12-17us -- keep this exact order.)
            for i, b in enumerate(order[1:]):
                nc.sync.dma_start(out=kts[b][:], in_=kT[b, :, :128 * kbs[b]])
                nc.sync.dma_start(out=qts[b][:], in_=qT[b])
                prev = order[i]
                nc.sync.dma_start(out=vts[prev][:],
                                  in_=vE[prev, :, :kbs[prev], :])
            bl = order[-1]
            nc.sync.dma_start(out=vts[bl][:], in_=vE[bl, :, :kbs[bl], :])

            # Pair stream: one iteration per (batch, key-tile). The two
            # heads' QKs are emitted adjacently as B0,A0,B1,A1: head A's
            # stationary lives in array rows 0:64 (tile_position (0,0)) and
            # head B's in rows 64:128 ((64,0), auto-derived from the SBUF
            # base partition), so consecutive matmuls land in different
            # row-strips and run CONCURRENTLY (observed dstart ~5ns),
            # nearly halving QK's PE occupancy.
            recs = {}
            pairs = []
            for bi, b in enumerate(order):
                kb = kbs[b]
                recs[b] = {"b": b, "kb": kb, "wts": [[None] * kb, [None] * kb],
                           "vt": vts[b]}
                pairs.extend((b, t) for t in range(kb))

            def emit_qk_pair(p, warm=0, warm_width=256, early=False):
                b, t = p
                if early:
                    # The acc PSUM banks sit idle until the first batch's
                    # A-V starts; borrowing them as a second pair-slot
                    # doubles the QK pipeline depth at the start so the PE
                    # runs real QKs back-to-back through the HAM activity
                    # window (2.4GHz flip) instead of idling exp-paced.
                    sc_a = acc_pool.tile([128, S], F32, tag="acc", name="esc")
                    sc_b = acc_pool.tile([128, S], F32, tag="acc", name="esc")
                else:
                    sc_a = sc_pool.tile([128, S], F32, tag="sc", name="sc")
                    sc_b = sc_pool.tile([128, S], F32, tag="sc", name="sc")
                junk(sc_a, warm, warm_width)
                # Head B first: its exp runs on the slower DVE leg
                # (Schraudolph 1224ns vs ACT 1114ns), so giving its QK
                # chunks the early strip slots balances the two
                # exp->QK->exp pipeline legs.
                for qc in range(2):
                    for h, sc in ((1, sc_b), (0, sc_a)):
                        base = 64 * h
                        nc.tensor.matmul(
                            sc[:, qc * 512:(qc + 1) * 512],
                            lhsT=kts[b][base:base + 64, t * 128:(t + 1) * 128],
                            rhs=qts[b][base:base + 64, qc * 512:(qc + 1) * 512],
                            start=True, stop=True,
                        )
                return (sc_a, sc_b)

            # Schraudolph constants for the DVE exp approximation:
            # bf16 bits of exp(s) ~= int16(A*s + B); A = 128/ln(2), B centers
            # the (1+f)/2^f mantissa error (max ~4% on weights, ~1% on the
            # softmax-averaged output). Head A's exp always runs on ScalarE
            # (true exp) and head B's on VectorE (Schraudolph) -- the two
            # run in parallel, one pair per ~1.25us.
            EXP_A = 184.6649652337873
            EXP_B = 16250.5

            def emit_exp_pair(p, scs):
                b, t = p
                for h in (0, 1):
                    wt = w_pool.tile([128, S], BF16, tag="w",
                                     name=f"w{b}_{t}_{h}")
                    if h == 1:
                        nc.vector.tensor_scalar(
                            wt[:].bitcast(mybir.dt.int16), scs[h][:],
                            EXP_A, EXP_B,
                            mybir.AluOpType.mult, mybir.AluOpType.add)
                    else:
                        nc.scalar.activation(wt[:], scs[h][:],
                                             mybir.ActivationFunctionType.Exp)
                    recs[b]["wts"][h][t] = wt

            def emit_av(p, h, t):
                """One A-V accumulation step: outT[65, S] += V_t.T @ W_t^T
                (two 512-col matmuls, one per PSUM bank half). The V tile is
                zero-padded to 128 stationary columns: the wider LDWEIGHTS
                (+53ns) is free PE-busy filler during the exp-gated stall and
                keeps the HAM idle fraction below the re-throttle threshold;
                output rows 65:128 are zeros and never read."""
                if t == 0:
                    p[f"acc{h}"] = acc_pool.tile([128, S], F32, tag="acc",
                                                 name=f"acc{p['b']}_{h}")
                kb = p["kb"]
                for half in range(2):
                    nc.tensor.matmul(
                        p[f"acc{h}"][:, half * 512:(half + 1) * 512],
                        lhsT=p["vt"][:, t, h * 128:h * 128 + 128],
                        rhs=p["wts"][h][t][:, half * 512:(half + 1) * 512],
                        start=(t == 0), stop=(t == kb - 1),
                    )
                if t == kb - 1:
                    epiq.append((p, h, 0, cur_pair[0]))
                    epiq.append((p, h, 1, cur_pair[0]))

            epiq = []  # deferred epilogue chunks: (rec, head, chunk, pair)
            epi_no = [0]
            cur_pair = [0]

            def emit_epi_chunk(min_age=2, on_act=None, dma_eng=None):
                """Ship one [65,512] chunk of a finished head's unnormalized
                sums: PSUM -> SBUF(bf16) -> HBM. At most one chunk per
                engine per pair iteration: a ~680ns copy fits in each exp
                engine's idle window (exp(i) can't start until QK(i) is
                done), so the copies ride for free instead of delaying the
                next exp -- emitting two on ONE engine measured +1.3us on
                that pair's cycle. A chunk is held until its A-V chain-stop
                matmul is at least 2 pairs old, else the strict-FIFO exp
                engine blocks on the not-yet-run accumulation."""
                if not epiq or epiq[0][3] > cur_pair[0] - min_age:
                    return
                p, h, c, _ = epiq.pop(0)
                if c == 0:
                    p[f"ot{h}"] = o_pool.tile([65, 2, 512], BF16, tag="ot",
                                              name="ot")
                ot = p[f"ot{h}"]
                src = p[f"acc{h}"][0:65, c * 512:(c + 1) * 512]
                if on_act is None:
                    on_act = epi_no[0] % 8 < 5
                epi_no[0] += 1
                if on_act:
                    nc.scalar.copy(ot[:, c, :], src)
                else:
                    nc.vector.tensor_scalar_mul(ot[:, c, :], src, 1.0)
                (dma_eng or nc.sync).dma_start(
                    out=o[p["b"], h][:, c * 512:(c + 1) * 512],
                    in_=ot[:, c, :])

            # Global stream, per pair i:
            #   epi chunk(s) -> exps(i) -> A-V drip (pairs <= i-lag) ->
            #   QK(i+1)
            # The drip sits BEFORE the next QK in the PE queue so the PE
            # chews A-V work while exps(i) run; QK(i+1) then starts as soon
            # as its score slots free (exp(i-1) with the 2-pair pipeline).
            # A-V for pair i only needs exp(i), so a lag of 1 pair is
            # enough in steady state; the first batch uses lag 2 because
            # the exps are still table-load/cold-clock delayed there.
            avq = []  # (batch record, head, t, pair idx) FIFO
            kb0 = kbs[order[0]]
            pend = emit_qk_pair(pairs[0], warm=10, warm_width=512)
            for i, p in enumerate(pairs):
                b, t = p
                rec = recs[b]
                cur_pair[0] = i
                if i < len(pairs) - 2:
                    # no epi copies in the last two pairs: they inflate the
                    # monotonic ACT/DVE counters that gate the final A-V
                    # matmuls (+0.6us on the tail)
                    emit_epi_chunk(on_act=True)
                    if len(epiq) >= 3 or i >= len(pairs) - 8:
                        # second chunk rides the DVE's window: drains the
                        # 4-chunk batch-boundary backlog in 2 pairs (the
                        # next batch's A-V chain waits on its acc banks'
                        # last epi read) and keeps the post-loop tail short
                        emit_epi_chunk(on_act=False)
                emit_exp_pair(p, pend)
                # Ramp notes: batch 0's AVs wait its V-tile DMA (~16.6us,
                # serial Sync issues) and block the FIFO ahead of batch 1's
                # first QK, pushing exp(pair 3) to ~20.5us. Every attempted
                # fix measured WORSE on max-core: deferring the AVs via
                # lag 4 (98.2us), QK-before-drip ordering in the ramp
                # (100.9us), DMA reorders (111-116us). The drip-first
                # lag-2 schedule below is the validated optimum.
                lag = 2 if i < kb0 else 1
                while avq and avq[0][3] <= i - lag:
                    emit_av(*avq.pop(0)[:3])
                if i + 1 < len(pairs):
                    pend = emit_qk_pair(pairs[i + 1],
                                        warm=2 if i < kb0 else 0,
                                        warm_width=256,
                                        early=i == 0)
                avq.append((rec, 0, t, i))
                avq.append((rec, 1, t, i))

            while avq:
                emit_av(*avq.pop(0)[:3])
            # Final drain: alternate the DMA issues between the Sync and
            # ScalarE HWDGE queues -- serially on Sync alone the last four
            # ~810ns issues are the critical-path tail after the copies.
            j = 0
            while epiq:
                emit_epi_chunk(min_age=-10**9, on_act=j % 2 == 0,
                               dma_eng=nc.scalar if j % 2 else nc.sync)
                j += 1
    _dedup_ldweights(nc)
    nc.compile()
    return nc


_NC_CACHE = {}


def _get_nc(kbs):
    key = tuple(kbs)
    if key not in _NC_CACHE:
        _NC_CACHE[key] = _build_nc(key)
    return _NC_CACHE[key]


def kernel(memory, query, b, seq_len):
    memory = np.asarray(memory)
    query = np.asarray(query)
    bias = np.asarray(b, dtype=np.float32)
    seq_len = np.asarray(seq_len).reshape(-1).astype(np.int64)

    sl = seq_len.copy()
    kbs = [int(min(KT, max(1, -(-int(s) // 128)))) if s > 0 else KT for s in sl]

    # emb[b, k] = exp(bias[k]) * valid; fully-masked batch -> plain softmax
    pos = np.arange(S)[None, :]
    valid = (pos < sl[:, None]) | (sl[:, None] == 0)
    emb = np.exp(bias)[None, :] * valid.astype(np.float32)  # [B, S]

    qh = (query.astype(np.float32) * (DH ** -0.5)).reshape(B, S, H, DH)
    kh = memory[:, :, :UNITS].astype(np.float32).reshape(B, S, H, DH)
    vh = memory[:, :, UNITS:].astype(np.float32).reshape(B, S, H, DH)
    vh = vh * emb[:, :, None, None]  # [B, S, H, DH] value rows pre-masked

    bf = ml_dtypes.bfloat16
    # [B, S, H, DH] -> [B, H, DH, S] transposed layouts
    qTfull = np.ascontiguousarray(qh.transpose(0, 2, 3, 1)).astype(bf)
    kTfull = np.ascontiguousarray(kh.transpose(0, 2, 3, 1)).astype(bf)
    # [B, S, H, DH] -> [B, (t p), H, DH] -> [B, 128, KT, H, DH]
    vtiles = np.ascontiguousarray(
        vh.reshape(B, KT, 128, H, DH).transpose(0, 2, 1, 3, 4)).astype(bf)
    embt = np.ascontiguousarray(
        emb.reshape(B, KT, 128).transpose(0, 2, 1)).astype(bf)  # [B, 128, KT]

    in_maps = []
    for c in range(N_CORES):
        hA, hB = 2 * c, 2 * c + 1
        qTc = np.concatenate([qTfull[:, hA], qTfull[:, hB]], axis=1)
        kTc = np.concatenate([kTfull[:, hA], kTfull[:, hB]], axis=1)
        vEc = np.zeros((B, 128, KT, 256), dtype=bf)
        vEc[..., 0:64] = vtiles[:, :, :, hA, :]
        vEc[..., 64] = embt
        vEc[..., 128:192] = vtiles[:, :, :, hB, :]
        vEc[..., 192] = embt
        in_maps.append({
            "qt": np.ascontiguousarray(qTc),
            "kt": np.ascontiguousarray(kTc),
            "vt": np.ascontiguousarray(vEc),
        })

    nc = _get_nc(kbs)
    res = run_bass_kernel_spmd(nc, in_maps, core_ids=list(range(N_CORES)))

    out = np.empty((B, S, UNITS), dtype=np.float32)
    for c in range(N_CORES):
        oc = np.asarray(res.results[c]["o"], dtype=np.float32)  # [B,2,65,S]
        num = oc[:, :, 0:64, :]                      # [B, 2, 64, S]
        den = oc[:, :, 64:65, :]                     # [B, 2, 1, S]
        core = (num / den).transpose(0, 3, 1, 2)     # [B, S, 2, 64]
        out[:, :, 128 * c:128 * (c + 1)] = core.reshape(B, S, 128)
    return out

